# revision 7
# baseline (speedup 1.0000x reference)
"""Trainium2 Bass kernel for CapsuleLayer dynamic routing.

Problem: u = einsum('bpe,pjed->bpjd', inp, W[0]) + b, then 3 routing
iterations (softmax over j, weighted sum over p, squash) -> vj [B,J,D].

Shapes: B=16, P=1024, J=32, Dp=D=64.  W is 512MB fp32 -> DMA dominated.

Strategy (8 NeuronCores):
 - Shard P across cores: 128 p's per core; all batches on every core.
 - Host packs W (bf16) as stacked pairs: rhs[pair] = [W_pe; W_po] with
   K=128=(2 p's x 64 e).  lhsT[pair] = block-diag([inp_pe, inp_po]) so one
   matmul computes u for 2 capsules with full contraction rows, M=32.
 - 4 pairs per PSUM round via col-group tile_position -> PSUM [128,512]
   slices, double buffered -> evictions (cast bf16) into SBUF
   u[(k,b), (g,d,j)].
 - Routing on-device: agreement mul on DVE writes tmp in (d,g,j) order;
   the d-reduction runs on the tensor engine as 64 accumulating
   identity matmuls per half (PSUM accumulate) -> a[(k,b),(g,j)].
   softmax (no max-sub; logits are small) via ACT exp; sum over p via
   0/1-masked (Delta) matmuls accumulating in PSUM; cross-core reduce via
   AllGather (bf16 partials) + Delta matmul over the 8 gathered shards.
   Final iteration's partials summed + squashed on host.
"""

import numpy as np
import ml_dtypes

import concourse.bass as bass
import concourse.tile as tile
from concourse import bacc, mybir
from concourse.bass_utils import run_bass_kernel_spmd

F32 = mybir.dt.float32
BF16 = mybir.dt.bfloat16
AX = mybir.AxisListType
AF = mybir.ActivationFunctionType

B = 16      # batch
J = 32      # output capsules
D = 64      # output capsule dim
E = 64      # input capsule dim
JD = J * D  # 2048


def build_program(n_cores: int, n_groups: int):
    """Build the SPMD Bass program. Per core: P_loc = 8*n_groups capsules."""
    G = n_groups
    ploc = 8 * G
    npair = ploc // 2
    nblk = npair // 2          # DMA blocks of 2 pairs
    UFREE = G * JD             # u free elements per partition
    GQ = 4                     # groups per agreement quarter
    NQ = G // GQ               # number of agreement quarters
    AQ = GQ * J                # agreement cols per quarter

    nc = bacc.Bacc("TRN2", target_bir_lowering=False, debug=False,
                   num_devices=n_cores)

    w_dram = nc.dram_tensor("w", [nblk, 128, 2 * JD], BF16, kind="ExternalInput")
    x_dram = nc.dram_tensor("x", [128, npair * 32], BF16, kind="ExternalInput")
    out_dram = nc.dram_tensor("out", [16, JD], F32, kind="ExternalOutput")

    with tile.TileContext(nc) as tc:
        with (
            tc.tile_pool(name="const", bufs=1) as constp,
            tc.tile_pool(name="wpool", bufs=2) as wpool,
            tc.tile_pool(name="upool", bufs=1) as upool,
            tc.tile_pool(name="work", bufs=2) as work,
            tc.tile_pool(name="small", bufs=1) as small,
            tc.tile_pool(name="pmain", bufs=2, space="PSUM") as pmain,
            tc.tile_pool(name="pacc", bufs=1, space="PSUM") as pacc,
            tc.tile_pool(name="pagr", bufs=1, space="PSUM") as pagr,
            tc.tile_pool(name="dram", bufs=1, space="DRAM") as dramp,
        ):
            # ---- static inputs -> SBUF ----
            x_sb = constp.tile([128, npair * 32], BF16)
            nc.sync.dma_start(x_sb[:], x_dram[:])
            # 0/1 mask constants built on-device (no DMA dependency):
            # delta[q, m] = (q % 16 == m); eye16[r, q] = (q % 16 == r);
            # eye128[q, m] = (q == m)
            I32 = mybir.dt.int32
            delta_sb = constp.tile([128, 16], BF16)
            qi = constp.tile([128, 128], I32)
            mi = constp.tile([128, 128], I32)
            ei = constp.tile([128, 128], F32)
            nc.gpsimd.iota(qi[:, :16], pattern=[[0, 16]], base=0,
                           channel_multiplier=1)
            nc.vector.tensor_scalar(qi[:, :16], qi[:, :16], 15, None,
                                    op0=mybir.AluOpType.bitwise_and)
            nc.gpsimd.iota(mi[:, :16], pattern=[[1, 16]], base=0,
                           channel_multiplier=0)
            nc.vector.tensor_tensor(ei[:, :16], qi[:, :16], mi[:, :16],
                                    op=mybir.AluOpType.is_equal)
            nc.vector.tensor_copy(delta_sb[:], ei[:, :16])
            eye16_sb = constp.tile([16, 128], BF16)
            nc.gpsimd.iota(qi[:16, :], pattern=[[1, 128]], base=0,
                           channel_multiplier=0)
            nc.vector.tensor_scalar(qi[:16, :], qi[:16, :], 15, None,
                                    op0=mybir.AluOpType.bitwise_and)
            nc.gpsimd.iota(mi[:16, :], pattern=[[0, 128]], base=0,
                           channel_multiplier=1)
            nc.vector.tensor_tensor(ei[:16, :], qi[:16, :], mi[:16, :],
                                    op=mybir.AluOpType.is_equal)
            nc.vector.tensor_copy(eye16_sb[:], ei[:16, :])
            eye128_sb = constp.tile([128, 128], BF16)
            nc.gpsimd.iota(qi[:, :], pattern=[[0, 128]], base=0,
                           channel_multiplier=1)
            nc.gpsimd.iota(mi[:, :], pattern=[[1, 128]], base=0,
                           channel_multiplier=0)
            nc.vector.tensor_tensor(ei[:, :], qi[:, :], mi[:, :],
                                    op=mybir.AluOpType.is_equal)
            nc.vector.tensor_copy(eye128_sb[:], ei[:, :])

            u_sb = upool.tile([128, UFREE], BF16)

            # ---- collective helpers ----
            n_cc = [0]

            def all_gather(src_bf16, nelem=JD):
                """AllGather a [16, nelem] bf16 shard -> [128, nelem] SBUF
                tile (rank r's shard at partitions 16r..16r+15)."""
                i = n_cc[0]
                n_cc[0] += 1
                cin = dramp.tile([16, nelem], BF16, tag=f"cin{i}",
                                 name=f"cc_in{i}")
                cout = dramp.tile([128, nelem], BF16, tag=f"cout{i}",
                                  addr_space="Shared" if n_cores > 4 else "Local",
                                  name=f"cc_out{i}")
                nc.sync.dma_start(cin[:], src_bf16[:])
                nc.gpsimd.collective_compute(
                    "AllGather", mybir.AluOpType.bypass,
                    replica_groups=[list(range(n_cores))],
                    ins=[cin.opt()], outs=[cout.opt()],
                )
                tag = "ag_a" if i % 2 == 0 else "ag_b"
                dst = small.tile([128, nelem], BF16, tag=tag, name=f"ag_dst{i}")
                nc.sync.dma_start(dst[:], cout[:])
                return dst

            # warmup collective to absorb first-CC setup cost (overlaps phase 1)
            warm = small.tile([16, 16], BF16, tag="warm", name="warm")
            nc.vector.memset(warm[:], 0.0)
            all_gather(warm, nelem=16)

            # ---- phase 1: stream W, matmul u, evict, accumulate s0 ----
            ps0 = pacc.tile([16, JD], F32, tag="pacc", name="ps0")
            for g in range(G):
                wtiles = []
                for half in range(2):
                    wt = wpool.tile([128, 2 * JD], BF16, tag="w",
                                    name=f"wt{g}_{half}", bufs=3)
                    eng = nc.sync if (2 * g + half) % 2 == 0 else nc.scalar
                    eng.dma_start(wt[:], w_dram[2 * g + half])
                    wtiles.append(wt)
                for q in range(4):
                    pm = pmain.tile([128, 512], F32, tag="pmain",
                                    name=f"pm{g}_{q}")
                    for cg in range(4):
                        pi = 4 * g + cg
                        lhsT = x_sb[:, pi * 32:(pi + 1) * 32]
                        half, cgl = divmod(cg, 2)
                        base = cgl * JD + q * 512
                        nc.tensor.matmul(
                            pm[32 * cg:32 * cg + 32, :],
                            lhsT,
                            wtiles[half][:, base:base + 512],
                            tile_position=(0, 32 * cg),
                        )
                    off = g * JD + q * 512
                    nc.scalar.copy(u_sb[:, off:off + 512], pm[:])
                    nc.tensor.matmul(
                        ps0[:, q * 512:(q + 1) * 512],
                        delta_sb[:],
                        u_sb[:, off:off + 512],
                        start=(g == 0), stop=(g == G - 1),
                        skip_group_check=True,
                    )

            # ---- squash + broadcast v to all 128 partitions (bf16) ----
            # s layout is [16, (d, j)]
            v_sb = constp.tile([128, JD], BF16)

            def reduce_partials(ag_sb, name):
                """Sum the 8 gathered [16,JD] bf16 partials -> PSUM [16,JD]."""
                ps = pacc.tile([16, JD], F32, tag="pacc", name=f"rp_{name}")
                for ns in range(4):
                    nc.tensor.matmul(
                        ps[:, ns * 512:(ns + 1) * 512],
                        delta_sb[:],
                        ag_sb[:, ns * 512:(ns + 1) * 512],
                        skip_group_check=True,
                    )
                s_sb = small.tile([16, JD], F32, tag="s_loc", name=f"s_{name}")
                nc.scalar.copy(s_sb[:], ps[:])
                return s_sb

            def squash_broadcast(s_sb, scale):
                # v = s*scale * sqrt(T)/(1+T), T = scale^2 * sum_d s^2
                # = s * [scale^2*sqrt(t_raw) * recip(1 + scale^2*t_raw)]
                s2t = small.tile([16, JD], F32, tag="s2t", name="s2t")
                nc.vector.tensor_mul(s2t[:], s_sb[:], s_sb[:])
                t = small.tile([16, J], F32, tag="t", name="t")
                nc.vector.reduce_sum(t[:], s2t[:].rearrange("p (d j) -> p j d", d=D),
                                     axis=AX.X)
                st = small.tile([16, J], F32, tag="st", name="st")
                nc.scalar.sqrt(st[:], t[:])
                den = small.tile([16, J], F32, tag="den", name="den")
                nc.vector.tensor_scalar(den[:], t[:], scale * scale, 1.0,
                                        op0=mybir.AluOpType.mult,
                                        op1=mybir.AluOpType.add)
                rec = small.tile([16, J], F32, tag="rec", name="rec")
                nc.vector.reciprocal(rec[:], den[:])
                f = small.tile([16, J], F32, tag="f", name="f")
                nc.vector.scalar_tensor_tensor(f[:], st[:], scale * scale, rec[:],
                                               op0=mybir.AluOpType.mult,
                                               op1=mybir.AluOpType.mult)
                v16 = small.tile([16, JD], BF16, tag="v16", name="v16")
                nc.vector.tensor_mul(
                    v16[:].rearrange("p (d j) -> p d j", d=D),
                    s_sb[:].rearrange("p (d j) -> p d j", d=D),
                    f[:].unsqueeze(1).broadcast_to([16, D, J]),
                )
                for q in range(4):
                    pv = pmain.tile([128, 512], F32, tag="pmain", name=f"pv{q}")
                    nc.tensor.matmul(
                        pv[:], eye16_sb[:],
                        v16[:, q * 512:(q + 1) * 512])
                    nc.scalar.copy(v_sb[:, q * 512:(q + 1) * 512], pv[:])

            # s0: evict PSUM accumulation -> bf16, AllGather, reduce, squash
            s0_loc = small.tile([16, JD], BF16, tag="s_out", name="s0_loc")
            nc.scalar.copy(s0_loc[:], ps0[:])
            ag_s0 = all_gather(s0_loc)
            s0 = reduce_partials(ag_s0, "s0")
            squash_broadcast(s0, 1.0 / J)

            # ---- routing iterations ----
            bij = constp.tile([128, G * J], F32)
            a_ps = None

            GC = 4                         # groups per cu chunk
            NCH = G // GC
            CH = GC * JD                   # u elems per chunk per partition

            for it in (1, 2):
                # agreement: tmp = u*v in (d, g, j) order per half (DVE),
                # then sum over d on the tensor engine (64 accumulating
                # identity matmuls into PSUM a[(k,b),(g,j)])
                a_ps = pagr.tile([128, G * J], F32, tag="pagr", name=f"a{it}")
                for hf in range(NQ):
                    tmp = work.tile([128, GQ * JD], BF16, tag="sc",
                                    name=f"agr{it}_{hf}")
                    u_h = u_sb[:, hf * GQ * JD:(hf + 1) * GQ * JD]
                    nc.vector.tensor_mul(
                        tmp[:].rearrange("p (d g j) -> p g d j", d=D, g=GQ),
                        u_h.rearrange("p (g d j) -> p g d j", g=GQ, d=D),
                        v_sb[:].rearrange("p (d j) -> p d j", d=D)
                            .unsqueeze(1).broadcast_to([128, GQ, D, J]),
                    )
                    for dd in range(D):
                        nc.tensor.matmul(
                            a_ps[:, hf * AQ:(hf + 1) * AQ],
                            eye128_sb[:],
                            tmp[:, dd * AQ:(dd + 1) * AQ],
                            start=(dd == 0), stop=(dd == D - 1),
                            skip_group_check=True,
                        )
                # bij += a ; softmax over j (no max subtraction: logits small)
                if it == 1:
                    nc.vector.tensor_copy(bij[:], a_ps[:])
                else:
                    nc.vector.tensor_add(bij[:], bij[:], a_ps[:])
                eh = small.tile([128, G * J], F32, tag="eh", name="eh")
                nc.scalar.activation(eh[:], bij[:], AF.Exp)
                eh3 = eh[:].rearrange("p (g j) -> p g j", g=G)
                se = small.tile([128, G], F32, tag="se", name="se")
                nc.vector.reduce_sum(se[:], eh3, axis=AX.X)
                re = small.tile([128, G], F32, tag="re", name="re")
                nc.vector.reciprocal(re[:], se[:])
                c_full = small.tile([128, G * J], BF16, tag="c_h",
                                    name="c_full")
                nc.vector.tensor_mul(
                    c_full[:].rearrange("p (g j) -> p g j", g=G), eh3,
                    re[:].unsqueeze(2).broadcast_to([128, G, J]))
                # weighted sum over p: cu = u*c then Delta matmuls -> ps
                ps = pacc.tile([16, JD], F32, tag="pacc", name=f"ps_it{it}")
                for h in range(NCH):
                    u_ch = u_sb[:, h * CH:(h + 1) * CH]
                    u4 = u_ch.rearrange("p (g d j) -> p g d j", g=GC, d=D)
                    cu = work.tile([128, CH], BF16, tag="sc", name=f"cu{it}_{h}")
                    nc.vector.tensor_mul(
                        cu[:].rearrange("p (g d j) -> p g d j", g=GC, d=D),
                        u4,
                        c_full[:, h * GC * J:(h + 1) * GC * J]
                            .rearrange("p (g j) -> p g j", g=GC)
                            .unsqueeze(2).broadcast_to([128, GC, D, J]),
                    )
                    for gg in range(GC):
                        for ns in range(4):
                            nc.tensor.matmul(
                                ps[:, ns * 512:(ns + 1) * 512],
                                delta_sb[:],
                                cu[:, gg * JD + ns * 512: gg * JD + (ns + 1) * 512],
                                start=(h == 0 and gg == 0),
                                stop=(h == NCH - 1 and gg == GC - 1),
                                skip_group_check=True,
                            )
                if it == 1:
                    s1_loc = small.tile([16, JD], BF16, tag="s_out",
                                        name="s1_loc")
                    nc.scalar.copy(s1_loc[:], ps[:])
                    ag_s1 = all_gather(s1_loc)
                    s1 = reduce_partials(ag_s1, "s1")
                    squash_broadcast(s1, 1.0)
                else:
                    s2_sb = small.tile([16, JD], F32, tag="s_loc", name="s2_sb")
                    nc.scalar.copy(s2_sb[:], ps[:])
                    nc.sync.dma_start(out_dram[:], s2_sb[:])

    nc.compile()
    return nc


def pack_inputs(inp, W, b, n_cores: int, n_groups: int):
    """Host-side packing -> per-core in_maps. W columns in (d, j) order."""
    P = inp.shape[1]
    G = n_groups
    ploc = 8 * G
    npair = ploc // 2
    nblk = npair // 2
    assert n_cores * ploc == P

    bf = ml_dtypes.bfloat16
    if b is not None and np.any(b):
        raise NotImplementedError("nonzero bias b is not supported")
    # W[0]: [P, J, E, D] -> [P, E, (D, J)]
    Wt = np.ascontiguousarray(W[0].transpose(0, 2, 3, 1)).reshape(P, E, JD)
    Wp = Wt.reshape(P // 2, 2 * E, JD)
    Wb = Wp.reshape(n_cores, nblk, 2, 2 * E, JD).transpose(0, 1, 3, 2, 4)
    w_dev = np.ascontiguousarray(Wb).reshape(n_cores, nblk, 128, 2 * JD).astype(bf)

    # x: [B, P, E] -> block diag lhsT [c, 128, npair*32]
    inpT = inp.transpose(1, 2, 0)          # [P, E, B]
    arr = inpT.reshape(n_cores, npair, 2, E, B)
    x_dev = np.zeros((n_cores, 2, E, npair, 2, 16), np.float32)
    x_dev[:, 0, :, :, 0, :] = arr[:, :, 0].transpose(0, 2, 1, 3)
    x_dev[:, 1, :, :, 1, :] = arr[:, :, 1].transpose(0, 2, 1, 3)
    x_dev = x_dev.reshape(n_cores, 128, npair * 32).astype(bf)

    in_maps = []
    for c in range(n_cores):
        in_maps.append({"w": w_dev[c], "x": x_dev[c]})
    return in_maps


def squash_np(x):
    s2 = np.sum(x * x, axis=-1, keepdims=True)
    return x * (s2 / (1.0 + s2)) / np.sqrt(s2)


_CACHE = {}


def kernel(inp: np.ndarray, W: np.ndarray, b: np.ndarray) -> np.ndarray:
    n_cores, n_groups = 8, 16
    inp = np.asarray(inp, dtype=np.float32)
    W = np.asarray(W, dtype=np.float32)
    b = np.asarray(b, dtype=np.float32)

    key = (n_cores, n_groups)
    if key not in _CACHE:
        _CACHE[key] = build_program(n_cores, n_groups)
    nc = _CACHE[key]

    in_maps = pack_inputs(inp, W, b, n_cores, n_groups)
    res = run_bass_kernel_spmd(nc, in_maps, core_ids=list(range(n_cores)))
    s2 = np.zeros((16, JD), np.float64)
    for r in res.results:
        s2 += r["out"].astype(np.float64)
    # s layout [16, (d, j)] -> [B, J, D]
    v = squash_np(s2.reshape(B, D, J).transpose(0, 2, 1))
    return v.astype(np.float32)


# revision 8
# speedup vs baseline: 1.2655x; 1.2655x over previous
"""Trainium2 Bass kernel for CapsuleLayer dynamic routing.

Problem: u = einsum('bpe,pjed->bpjd', inp, W[0]) + b, then 3 routing
iterations (softmax over j, weighted sum over p, squash) -> vj [B,J,D].

Shapes: B=16, P=1024, J=32, Dp=D=64.  W is 512MB fp32 -> DMA dominated.

Strategy (8 NeuronCores):
 - Shard P across cores: 128 p's per core; all batches on every core.
 - Host packs W (bf16) as stacked pairs: rhs[pair] = [W_pe; W_po] with
   K=128=(2 p's x 64 e).  lhsT[pair] = block-diag([inp_pe, inp_po]) so one
   matmul computes u for 2 capsules with full contraction rows, M=32.
 - 4 pairs per PSUM round via col-group tile_position -> PSUM [128,512]
   slices, double buffered -> evictions (cast bf16) into SBUF
   u[(k,b), (g,d,j)].
 - Routing on-device: agreement mul on DVE writes tmp in (d,g,j) order;
   the d-reduction runs on the tensor engine as 64 accumulating
   identity matmuls per half (PSUM accumulate) -> a[(k,b),(g,j)].
   softmax (no max-sub; logits are small) via ACT exp; sum over p via
   0/1-masked (Delta) matmuls accumulating in PSUM; cross-core reduce via
   AllGather (bf16 partials) + Delta matmul over the 8 gathered shards.
   Final iteration's partials summed + squashed on host.
"""

import numpy as np
import ml_dtypes

import concourse.bass as bass
import concourse.tile as tile
from concourse import bacc, mybir
from concourse.bass_utils import run_bass_kernel_spmd

F32 = mybir.dt.float32
BF16 = mybir.dt.bfloat16
AX = mybir.AxisListType
AF = mybir.ActivationFunctionType

B = 16      # batch
J = 32      # output capsules
D = 64      # output capsule dim
E = 64      # input capsule dim
JD = J * D  # 2048


def build_program(n_cores: int, n_groups: int):
    """Build the SPMD Bass program. Per core: P_loc = 8*n_groups capsules."""
    G = n_groups
    ploc = 8 * G
    npair = ploc // 2
    nblk = npair // 2          # DMA blocks of 2 pairs
    UFREE = G * JD             # u free elements per partition
    GQ = 4                     # groups per agreement quarter
    NQ = G // GQ               # number of agreement quarters
    AQ = GQ * J                # agreement cols per quarter

    nc = bacc.Bacc("TRN2", target_bir_lowering=False, debug=False,
                   num_devices=n_cores)

    w_dram = nc.dram_tensor("w", [nblk, 128, 2 * JD], BF16, kind="ExternalInput")
    x_dram = nc.dram_tensor("x", [128, npair * 32], BF16, kind="ExternalInput")
    out_dram = nc.dram_tensor("out", [16, JD], F32, kind="ExternalOutput")

    with tile.TileContext(nc) as tc:
        with (
            tc.tile_pool(name="const", bufs=1) as constp,
            tc.tile_pool(name="wpool", bufs=2) as wpool,
            tc.tile_pool(name="upool", bufs=1) as upool,
            tc.tile_pool(name="work", bufs=2) as work,
            tc.tile_pool(name="small", bufs=1) as small,
            tc.tile_pool(name="pmain", bufs=2, space="PSUM") as pmain,
            tc.tile_pool(name="pacc", bufs=1, space="PSUM") as pacc,
            tc.tile_pool(name="dram", bufs=1, space="DRAM") as dramp,
        ):
            # ---- static inputs -> SBUF ----
            x_sb = constp.tile([128, npair * 32], BF16)
            nc.sync.dma_start(x_sb[:], x_dram[:])
            # 0/1 mask constants built on-device (no DMA dependency):
            # delta[q, m] = (q % 16 == m); eye16[r, q] = (q % 16 == r);
            # eye128[q, m] = (q == m)
            I32 = mybir.dt.int32
            delta_sb = constp.tile([128, 16], BF16)
            qi = constp.tile([128, 128], I32)
            mi = constp.tile([128, 128], I32)
            ei = constp.tile([128, 128], F32)
            nc.gpsimd.iota(qi[:, :16], pattern=[[0, 16]], base=0,
                           channel_multiplier=1)
            nc.vector.tensor_scalar(qi[:, :16], qi[:, :16], 15, None,
                                    op0=mybir.AluOpType.bitwise_and)
            nc.gpsimd.iota(mi[:, :16], pattern=[[1, 16]], base=0,
                           channel_multiplier=0)
            nc.vector.tensor_tensor(ei[:, :16], qi[:, :16], mi[:, :16],
                                    op=mybir.AluOpType.is_equal)
            nc.vector.tensor_copy(delta_sb[:], ei[:, :16])
            eye16_sb = constp.tile([16, 128], BF16)
            nc.gpsimd.iota(qi[:16, :], pattern=[[1, 128]], base=0,
                           channel_multiplier=0)
            nc.vector.tensor_scalar(qi[:16, :], qi[:16, :], 15, None,
                                    op0=mybir.AluOpType.bitwise_and)
            nc.gpsimd.iota(mi[:16, :], pattern=[[0, 128]], base=0,
                           channel_multiplier=1)
            nc.vector.tensor_tensor(ei[:16, :], qi[:16, :], mi[:16, :],
                                    op=mybir.AluOpType.is_equal)
            nc.vector.tensor_copy(eye16_sb[:], ei[:16, :])
            eye128_sb = constp.tile([128, 128], BF16)
            nc.gpsimd.iota(qi[:, :], pattern=[[0, 128]], base=0,
                           channel_multiplier=1)
            nc.gpsimd.iota(mi[:, :], pattern=[[1, 128]], base=0,
                           channel_multiplier=0)
            nc.vector.tensor_tensor(ei[:, :], qi[:, :], mi[:, :],
                                    op=mybir.AluOpType.is_equal)
            nc.vector.tensor_copy(eye128_sb[:], ei[:, :])

            u_sb = upool.tile([128, UFREE], BF16)

            # ---- collective helpers ----
            n_cc = [0]

            def all_gather(src_bf16, nelem=JD):
                """AllGather a [16, nelem] bf16 shard -> [128, nelem] SBUF
                tile (rank r's shard at partitions 16r..16r+15)."""
                i = n_cc[0]
                n_cc[0] += 1
                cin = dramp.tile([16, nelem], BF16, tag=f"cin{i}",
                                 name=f"cc_in{i}")
                cout = dramp.tile([128, nelem], BF16, tag=f"cout{i}",
                                  addr_space="Shared" if n_cores > 4 else "Local",
                                  name=f"cc_out{i}")
                nc.gpsimd.dma_start(cin[:], src_bf16[:])
                nc.gpsimd.collective_compute(
                    "AllGather", mybir.AluOpType.bypass,
                    replica_groups=[list(range(n_cores))],
                    ins=[cin.opt()], outs=[cout.opt()],
                )
                tag = "ag_a" if i % 2 == 0 else "ag_b"
                dst = small.tile([128, nelem], BF16, tag=tag, name=f"ag_dst{i}")
                nc.gpsimd.dma_start(dst[:], cout[:])
                return dst

            # warmup collective to absorb first-CC setup cost (overlaps phase 1)
            warm = small.tile([16, 16], BF16, tag="warm", name="warm")
            nc.vector.memset(warm[:], 0.0)
            all_gather(warm, nelem=16)

            # ---- phase 1: stream W, matmul u, evict, accumulate s0 ----
            ps0 = pacc.tile([16, JD], F32, tag="pacc", name="ps0")
            for g in range(G):
                wtiles = []
                for half in range(2):
                    wt = wpool.tile([128, 2 * JD], BF16, tag="w",
                                    name=f"wt{g}_{half}", bufs=4)
                    eng = nc.sync if (2 * g + half) % 2 == 0 else nc.scalar
                    eng.dma_start(wt[:], w_dram[2 * g + half])
                    wtiles.append(wt)
                for hn in range(2):
                    pm = pmain.tile([128, 1024], F32, tag="pmain",
                                    name=f"pm{g}_{hn}")
                    for ns in range(2):
                        for cg in range(4):
                            pi = 4 * g + cg
                            lhsT = x_sb[:, pi * 32:(pi + 1) * 32]
                            half, cgl = divmod(cg, 2)
                            base = cgl * JD + hn * 1024
                            nc.tensor.matmul(
                                pm[32 * cg:32 * cg + 32, ns * 512:(ns + 1) * 512],
                                lhsT,
                                wtiles[half][:, base + ns * 512:
                                             base + (ns + 1) * 512],
                                tile_position=(0, 32 * cg),
                            )
                    off = g * JD + hn * 1024
                    nc.scalar.copy(u_sb[:, off:off + 1024], pm[:])
                    for ns in range(2):
                        nc.tensor.matmul(
                            ps0[:, hn * 1024 + ns * 512: hn * 1024 + (ns + 1) * 512],
                            delta_sb[:],
                            u_sb[:, off + ns * 512: off + (ns + 1) * 512],
                            start=(g == 0), stop=(g == G - 1),
                            skip_group_check=True,
                        )

            # ---- squash + broadcast v to all 128 partitions (bf16) ----
            # s layout is [16, (d, j)]
            v_sb = constp.tile([128, JD], BF16)

            def reduce_partials(ag_sb, name):
                """Sum the 8 gathered [16,JD] bf16 partials -> PSUM [16,JD]."""
                ps = pacc.tile([16, JD], F32, tag="pacc", name=f"rp_{name}")
                for ns in range(4):
                    nc.tensor.matmul(
                        ps[:, ns * 512:(ns + 1) * 512],
                        delta_sb[:],
                        ag_sb[:, ns * 512:(ns + 1) * 512],
                        skip_group_check=True,
                    )
                s_sb = small.tile([16, JD], F32, tag="s_loc", name=f"s_{name}")
                nc.scalar.copy(s_sb[:], ps[:])
                return s_sb

            def squash_broadcast(s_sb, scale):
                # v = s*scale * sqrt(T)/(1+T), T = scale^2 * sum_d s^2
                # = s * [scale^2*sqrt(t_raw) * recip(1 + scale^2*t_raw)]
                s2t = small.tile([16, JD], F32, tag="s2t", name="s2t")
                nc.vector.tensor_mul(s2t[:], s_sb[:], s_sb[:])
                t = small.tile([16, J], F32, tag="t", name="t")
                nc.vector.reduce_sum(t[:], s2t[:].rearrange("p (d j) -> p j d", d=D),
                                     axis=AX.X)
                st = small.tile([16, J], F32, tag="st", name="st")
                nc.scalar.sqrt(st[:], t[:])
                den = small.tile([16, J], F32, tag="den", name="den")
                nc.vector.tensor_scalar(den[:], t[:], scale * scale, 1.0,
                                        op0=mybir.AluOpType.mult,
                                        op1=mybir.AluOpType.add)
                rec = small.tile([16, J], F32, tag="rec", name="rec")
                nc.vector.reciprocal(rec[:], den[:])
                f = small.tile([16, J], F32, tag="f", name="f")
                nc.vector.scalar_tensor_tensor(f[:], st[:], scale * scale, rec[:],
                                               op0=mybir.AluOpType.mult,
                                               op1=mybir.AluOpType.mult)
                v16 = small.tile([16, JD], BF16, tag="v16", name="v16")
                nc.vector.tensor_mul(
                    v16[:].rearrange("p (d j) -> p d j", d=D),
                    s_sb[:].rearrange("p (d j) -> p d j", d=D),
                    f[:].unsqueeze(1).broadcast_to([16, D, J]),
                )
                for hn in range(2):
                    pv = pmain.tile([128, 1024], F32, tag="pmain", name=f"pv{hn}")
                    for ns in range(2):
                        nc.tensor.matmul(
                            pv[:, ns * 512:(ns + 1) * 512], eye16_sb[:],
                            v16[:, hn * 1024 + ns * 512: hn * 1024 + (ns + 1) * 512])
                    nc.scalar.copy(v_sb[:, hn * 1024:(hn + 1) * 1024], pv[:])

            # s0: evict PSUM accumulation -> bf16, AllGather, reduce, squash
            s0_loc = small.tile([16, JD], BF16, tag="s_out", name="s0_loc")
            nc.scalar.copy(s0_loc[:], ps0[:])
            ag_s0 = all_gather(s0_loc)
            s0 = reduce_partials(ag_s0, "s0")
            squash_broadcast(s0, 1.0 / J)

            # ---- routing iterations ----
            bij = constp.tile([128, G * J], F32)
            a_ps = None

            GC = 4                         # groups per cu chunk
            NCH = G // GC
            CH = GC * JD                   # u elems per chunk per partition

            for it in (1, 2):
                # agreement: tmp = u*v in (d, g, j) order per half (DVE),
                # then sum over d on the tensor engine (64 accumulating
                # identity matmuls into PSUM a[(k,b),(g,j)])
                a_pm = pmain.tile([128, 1024], F32, tag="pmain", name=f"a{it}")
                a_ps = a_pm[:, :G * J]
                for hf in range(NQ):
                    tmp = work.tile([128, GQ * JD], BF16, tag="sc",
                                    name=f"agr{it}_{hf}")
                    u_h = u_sb[:, hf * GQ * JD:(hf + 1) * GQ * JD]
                    nc.vector.tensor_mul(
                        tmp[:].rearrange("p (d g j) -> p g d j", d=D, g=GQ),
                        u_h.rearrange("p (g d j) -> p g d j", g=GQ, d=D),
                        v_sb[:].rearrange("p (d j) -> p d j", d=D)
                            .unsqueeze(1).broadcast_to([128, GQ, D, J]),
                    )
                    for dd in range(D):
                        nc.tensor.matmul(
                            a_ps[:, hf * AQ:(hf + 1) * AQ],
                            eye128_sb[:],
                            tmp[:, dd * AQ:(dd + 1) * AQ],
                            start=(dd == 0), stop=(dd == D - 1),
                            skip_group_check=True,
                        )
                # bij += a ; softmax over j (no max subtraction: logits small)
                if it == 1:
                    nc.vector.tensor_copy(bij[:], a_ps[:])
                else:
                    nc.vector.tensor_add(bij[:], bij[:], a_ps[:])
                eh = small.tile([128, G * J], F32, tag="eh", name="eh")
                nc.scalar.activation(eh[:], bij[:], AF.Exp)
                eh3 = eh[:].rearrange("p (g j) -> p g j", g=G)
                se = small.tile([128, G], F32, tag="se", name="se")
                nc.vector.reduce_sum(se[:], eh3, axis=AX.X)
                re = small.tile([128, G], F32, tag="re", name="re")
                nc.vector.reciprocal(re[:], se[:])
                c_full = small.tile([128, G * J], BF16, tag="c_h",
                                    name="c_full")
                nc.vector.tensor_mul(
                    c_full[:].rearrange("p (g j) -> p g j", g=G), eh3,
                    re[:].unsqueeze(2).broadcast_to([128, G, J]))
                # weighted sum over p: cu = u*c then Delta matmuls -> ps
                ps = pacc.tile([16, JD], F32, tag="pacc", name=f"ps_it{it}")
                for h in range(NCH):
                    u_ch = u_sb[:, h * CH:(h + 1) * CH]
                    u4 = u_ch.rearrange("p (g d j) -> p g d j", g=GC, d=D)
                    cu = work.tile([128, CH], BF16, tag="sc", name=f"cu{it}_{h}")
                    nc.vector.tensor_mul(
                        cu[:].rearrange("p (g d j) -> p g d j", g=GC, d=D),
                        u4,
                        c_full[:, h * GC * J:(h + 1) * GC * J]
                            .rearrange("p (g j) -> p g j", g=GC)
                            .unsqueeze(2).broadcast_to([128, GC, D, J]),
                    )
                    for gg in range(GC):
                        for ns in range(4):
                            nc.tensor.matmul(
                                ps[:, ns * 512:(ns + 1) * 512],
                                delta_sb[:],
                                cu[:, gg * JD + ns * 512: gg * JD + (ns + 1) * 512],
                                start=(h == 0 and gg == 0),
                                stop=(h == NCH - 1 and gg == GC - 1),
                                skip_group_check=True,
                            )
                if it == 1:
                    s1_loc = small.tile([16, JD], BF16, tag="s_out",
                                        name="s1_loc")
                    nc.scalar.copy(s1_loc[:], ps[:])
                    ag_s1 = all_gather(s1_loc)
                    s1 = reduce_partials(ag_s1, "s1")
                    squash_broadcast(s1, 1.0)
                else:
                    s2_sb = small.tile([16, JD], F32, tag="s_loc", name="s2_sb")
                    nc.scalar.copy(s2_sb[:], ps[:])
                    nc.sync.dma_start(out_dram[:], s2_sb[:])

    nc.compile()
    return nc


def pack_inputs(inp, W, b, n_cores: int, n_groups: int):
    """Host-side packing -> per-core in_maps. W columns in (d, j) order."""
    P = inp.shape[1]
    G = n_groups
    ploc = 8 * G
    npair = ploc // 2
    nblk = npair // 2
    assert n_cores * ploc == P

    bf = ml_dtypes.bfloat16
    if b is not None and np.any(b):
        raise NotImplementedError("nonzero bias b is not supported")
    # W[0]: [P, J, E, D] -> [P, E, (D, J)]
    Wt = np.ascontiguousarray(W[0].transpose(0, 2, 3, 1)).reshape(P, E, JD)
    Wp = Wt.reshape(P // 2, 2 * E, JD)
    Wb = Wp.reshape(n_cores, nblk, 2, 2 * E, JD).transpose(0, 1, 3, 2, 4)
    w_dev = np.ascontiguousarray(Wb).reshape(n_cores, nblk, 128, 2 * JD).astype(bf)

    # x: [B, P, E] -> block diag lhsT [c, 128, npair*32]
    inpT = inp.transpose(1, 2, 0)          # [P, E, B]
    arr = inpT.reshape(n_cores, npair, 2, E, B)
    x_dev = np.zeros((n_cores, 2, E, npair, 2, 16), np.float32)
    x_dev[:, 0, :, :, 0, :] = arr[:, :, 0].transpose(0, 2, 1, 3)
    x_dev[:, 1, :, :, 1, :] = arr[:, :, 1].transpose(0, 2, 1, 3)
    x_dev = x_dev.reshape(n_cores, 128, npair * 32).astype(bf)

    in_maps = []
    for c in range(n_cores):
        in_maps.append({"w": w_dev[c], "x": x_dev[c]})
    return in_maps


def squash_np(x):
    s2 = np.sum(x * x, axis=-1, keepdims=True)
    return x * (s2 / (1.0 + s2)) / np.sqrt(s2)


_CACHE = {}


def kernel(inp: np.ndarray, W: np.ndarray, b: np.ndarray) -> np.ndarray:
    n_cores, n_groups = 8, 16
    inp = np.asarray(inp, dtype=np.float32)
    W = np.asarray(W, dtype=np.float32)
    b = np.asarray(b, dtype=np.float32)

    key = (n_cores, n_groups)
    if key not in _CACHE:
        _CACHE[key] = build_program(n_cores, n_groups)
    nc = _CACHE[key]

    in_maps = pack_inputs(inp, W, b, n_cores, n_groups)
    res = run_bass_kernel_spmd(nc, in_maps, core_ids=list(range(n_cores)))
    s2 = np.zeros((16, JD), np.float64)
    for r in res.results:
        s2 += r["out"].astype(np.float64)
    # s layout [16, (d, j)] -> [B, J, D]
    v = squash_np(s2.reshape(B, D, J).transpose(0, 2, 1))
    return v.astype(np.float32)


# revision 9
# speedup vs baseline: 1.3024x; 1.0291x over previous
"""Trainium2 Bass kernel for CapsuleLayer dynamic routing.

Problem: u = einsum('bpe,pjed->bpjd', inp, W[0]) + b, then 3 routing
iterations (softmax over j, weighted sum over p, squash) -> vj [B,J,D].

Shapes: B=16, P=1024, J=32, Dp=D=64.  W is 512MB fp32 -> DMA dominated.

Strategy (8 NeuronCores):
 - Shard P across cores: 128 p's per core; all batches on every core.
 - Host packs W (bf16) as stacked pairs: rhs[pair] = [W_pe; W_po] with
   K=128=(2 p's x 64 e).  lhsT[pair] = block-diag([inp_pe, inp_po]) so one
   matmul computes u for 2 capsules with full contraction rows, M=32.
 - 4 pairs per PSUM round via col-group tile_position -> PSUM [128,512]
   slices, double buffered -> evictions (cast bf16) into SBUF
   u[(k,b), (g,d,j)].
 - Routing on-device: agreement mul on DVE writes tmp in (d,g,j) order;
   the d-reduction runs on the tensor engine as 64 accumulating
   identity matmuls per half (PSUM accumulate) -> a[(k,b),(g,j)].
   softmax (no max-sub; logits are small) via ACT exp; sum over p via
   0/1-masked (Delta) matmuls accumulating in PSUM; cross-core reduce via
   AllGather (bf16 partials) + Delta matmul over the 8 gathered shards.
   Final iteration's partials summed + squashed on host.
"""

import numpy as np
import ml_dtypes

import concourse.bass as bass
import concourse.tile as tile
from concourse import bacc, mybir
from concourse.bass_utils import run_bass_kernel_spmd

F32 = mybir.dt.float32
BF16 = mybir.dt.bfloat16
AX = mybir.AxisListType
AF = mybir.ActivationFunctionType

B = 16      # batch
J = 32      # output capsules
D = 64      # output capsule dim
E = 64      # input capsule dim
JD = J * D  # 2048


def build_program(n_cores: int, n_groups: int):
    """Build the SPMD Bass program. Per core: P_loc = 8*n_groups capsules."""
    G = n_groups
    ploc = 8 * G
    npair = ploc // 2
    nblk = npair // 2          # DMA blocks of 2 pairs
    UFREE = G * JD             # u free elements per partition
    GQ = 4                     # groups per agreement quarter
    NQ = G // GQ               # number of agreement quarters
    AQ = GQ * J                # agreement cols per quarter

    nc = bacc.Bacc("TRN2", target_bir_lowering=False, debug=False,
                   num_devices=n_cores)

    w_dram = nc.dram_tensor("w", [nblk, 128, 2 * JD], BF16, kind="ExternalInput")
    x_dram = nc.dram_tensor("x", [128, npair * 32], BF16, kind="ExternalInput")
    out_dram = nc.dram_tensor("out", [16, JD], F32, kind="ExternalOutput")

    with tile.TileContext(nc) as tc:
        with (
            tc.tile_pool(name="const", bufs=1) as constp,
            tc.tile_pool(name="wpool", bufs=2) as wpool,
            tc.tile_pool(name="upool", bufs=1) as upool,
            tc.tile_pool(name="work", bufs=3) as work,
            tc.tile_pool(name="small", bufs=1) as small,
            tc.tile_pool(name="pmain", bufs=2, space="PSUM") as pmain,
            tc.tile_pool(name="pacc", bufs=1, space="PSUM") as pacc,
            tc.tile_pool(name="dram", bufs=1, space="DRAM") as dramp,
        ):
            # ---- static inputs -> SBUF ----
            x_sb = constp.tile([128, npair * 32], BF16)
            nc.sync.dma_start(x_sb[:], x_dram[:])
            # 0/1 mask constants built on-device (no DMA dependency):
            # delta[q, m] = (q % 16 == m); eye16[r, q] = (q % 16 == r);
            # eye128[q, m] = (q == m)
            I32 = mybir.dt.int32
            delta_sb = constp.tile([128, 16], BF16)
            qi = constp.tile([128, 128], I32)
            mi = constp.tile([128, 128], I32)
            ei = constp.tile([128, 128], F32)
            nc.gpsimd.iota(qi[:, :16], pattern=[[0, 16]], base=0,
                           channel_multiplier=1)
            nc.vector.tensor_scalar(qi[:, :16], qi[:, :16], 15, None,
                                    op0=mybir.AluOpType.bitwise_and)
            nc.gpsimd.iota(mi[:, :16], pattern=[[1, 16]], base=0,
                           channel_multiplier=0)
            nc.vector.tensor_tensor(ei[:, :16], qi[:, :16], mi[:, :16],
                                    op=mybir.AluOpType.is_equal)
            nc.vector.tensor_copy(delta_sb[:], ei[:, :16])
            eye16_sb = constp.tile([16, 128], BF16)
            nc.gpsimd.iota(qi[:16, :], pattern=[[1, 128]], base=0,
                           channel_multiplier=0)
            nc.vector.tensor_scalar(qi[:16, :], qi[:16, :], 15, None,
                                    op0=mybir.AluOpType.bitwise_and)
            nc.gpsimd.iota(mi[:16, :], pattern=[[0, 128]], base=0,
                           channel_multiplier=1)
            nc.vector.tensor_tensor(ei[:16, :], qi[:16, :], mi[:16, :],
                                    op=mybir.AluOpType.is_equal)
            nc.vector.tensor_copy(eye16_sb[:], ei[:16, :])
            eye128_sb = constp.tile([128, 128], BF16)
            nc.gpsimd.iota(qi[:, :], pattern=[[0, 128]], base=0,
                           channel_multiplier=1)
            nc.gpsimd.iota(mi[:, :], pattern=[[1, 128]], base=0,
                           channel_multiplier=0)
            nc.vector.tensor_tensor(ei[:, :], qi[:, :], mi[:, :],
                                    op=mybir.AluOpType.is_equal)
            nc.vector.tensor_copy(eye128_sb[:], ei[:, :])

            u_sb = upool.tile([128, UFREE], BF16)

            # ---- collective helpers ----
            n_cc = [0]

            def all_gather(src_bf16, nelem=JD):
                """AllGather a [16, nelem] bf16 shard -> [128, nelem] SBUF
                tile (rank r's shard at partitions 16r..16r+15)."""
                i = n_cc[0]
                n_cc[0] += 1
                cin = dramp.tile([16, nelem], BF16, tag=f"cin{i}",
                                 name=f"cc_in{i}")
                cout = dramp.tile([128, nelem], BF16, tag=f"cout{i}",
                                  addr_space="Shared" if n_cores > 4 else "Local",
                                  name=f"cc_out{i}")
                nc.gpsimd.dma_start(cin[:], src_bf16[:])
                nc.gpsimd.collective_compute(
                    "AllGather", mybir.AluOpType.bypass,
                    replica_groups=[list(range(n_cores))],
                    ins=[cin.opt()], outs=[cout.opt()],
                )
                tag = "ag_a" if i % 2 == 0 else "ag_b"
                dst = small.tile([128, nelem], BF16, tag=tag, name=f"ag_dst{i}")
                nc.gpsimd.dma_start(dst[:], cout[:])
                return dst

            # warmup collective to absorb first-CC setup cost (overlaps phase 1)
            warm = small.tile([16, 16], BF16, tag="warm", name="warm")
            nc.vector.memset(warm[:], 0.0)
            all_gather(warm, nelem=16)

            # ---- phase 1: stream W, matmul u, evict, accumulate s0 ----
            # s0 split: groups [0, GS0) -> early AllGather (hidden under the
            # phase-1 tail); groups [GS0, G) -> second AllGather at the end.
            GS0 = G - 4
            ag_s0 = []
            ps0 = pacc.tile([16, JD], F32, tag="pacc", name="ps0_a")
            for g in range(G):
                if g == GS0:
                    s0a = small.tile([16, JD], BF16, tag="s_out", name="s0a")
                    nc.scalar.copy(s0a[:], ps0[:])
                    ag_s0.append(all_gather(s0a))
                    ps0 = pacc.tile([16, JD], F32, tag="pacc", name="ps0_b")
                wtiles = []
                for half in range(2):
                    wt = wpool.tile([128, 2 * JD], BF16, tag="w",
                                    name=f"wt{g}_{half}", bufs=4)
                    eng = nc.sync if (2 * g + half) % 2 == 0 else nc.scalar
                    eng.dma_start(wt[:], w_dram[2 * g + half])
                    wtiles.append(wt)
                for hn in range(2):
                    pm = pmain.tile([128, 1024], F32, tag="pmain",
                                    name=f"pm{g}_{hn}")
                    for ns in range(2):
                        for cg in range(4):
                            pi = 4 * g + cg
                            lhsT = x_sb[:, pi * 32:(pi + 1) * 32]
                            half, cgl = divmod(cg, 2)
                            base = cgl * JD + hn * 1024
                            nc.tensor.matmul(
                                pm[32 * cg:32 * cg + 32, ns * 512:(ns + 1) * 512],
                                lhsT,
                                wtiles[half][:, base + ns * 512:
                                             base + (ns + 1) * 512],
                                tile_position=(0, 32 * cg),
                            )
                    off = g * JD + hn * 1024
                    nc.scalar.copy(u_sb[:, off:off + 1024], pm[:])
                    for ns in range(2):
                        nc.tensor.matmul(
                            ps0[:, hn * 1024 + ns * 512: hn * 1024 + (ns + 1) * 512],
                            delta_sb[:],
                            u_sb[:, off + ns * 512: off + (ns + 1) * 512],
                            start=(g == 0 or g == GS0),
                            stop=(g == GS0 - 1 or g == G - 1),
                            skip_group_check=True,
                        )

            # ---- squash + broadcast v to all 128 partitions (bf16) ----
            # s layout is [16, (d, j)]
            v_sb = constp.tile([128, JD], BF16)

            def reduce_partials(ag_list, name):
                """Sum the gathered [128,JD] bf16 partial stacks -> bf16 s."""
                ps = pacc.tile([16, JD], F32, tag="pacc", name=f"rp_{name}")
                for ai, ag_sb in enumerate(ag_list):
                    for ns in range(4):
                        nc.tensor.matmul(
                            ps[:, ns * 512:(ns + 1) * 512],
                            delta_sb[:],
                            ag_sb[:, ns * 512:(ns + 1) * 512],
                            start=(ai == 0), stop=(ai == len(ag_list) - 1),
                            skip_group_check=True,
                        )
                s_bf = small.tile([16, JD], BF16, tag="s_bf", name=f"s_{name}")
                nc.scalar.copy(s_bf[:], ps[:])
                return s_bf

            def squash_broadcast(s_bf, scale):
                # v = s*scale * sqrt(T)/(1+T), T = scale^2 * sum_d s^2
                # = s * [scale^2*sqrt(t_raw) * recip(1 + scale^2*t_raw)]
                s2t = small.tile([16, JD], BF16, tag="s2t", name="s2t")
                nc.vector.tensor_mul(s2t[:], s_bf[:], s_bf[:])
                t = small.tile([16, J], F32, tag="t", name="t")
                nc.vector.reduce_sum(t[:], s2t[:].rearrange("p (d j) -> p j d", d=D),
                                     axis=AX.X)
                st = small.tile([16, J], F32, tag="st", name="st")
                nc.scalar.sqrt(st[:], t[:])
                den = small.tile([16, J], F32, tag="den", name="den")
                nc.vector.tensor_scalar(den[:], t[:], scale * scale, 1.0,
                                        op0=mybir.AluOpType.mult,
                                        op1=mybir.AluOpType.add)
                rec = small.tile([16, J], F32, tag="rec", name="rec")
                nc.vector.reciprocal(rec[:], den[:])
                f = small.tile([16, J], BF16, tag="f", name="f")
                nc.vector.scalar_tensor_tensor(f[:], st[:], scale * scale, rec[:],
                                               op0=mybir.AluOpType.mult,
                                               op1=mybir.AluOpType.mult)
                v16 = small.tile([16, JD], BF16, tag="v16", name="v16")
                nc.vector.tensor_mul(
                    v16[:].rearrange("p (d j) -> p d j", d=D),
                    s_bf[:].rearrange("p (d j) -> p d j", d=D),
                    f[:].unsqueeze(1).broadcast_to([16, D, J]),
                )
                for hn in range(2):
                    pv = pmain.tile([128, 1024], F32, tag="pmain", name=f"pv{hn}")
                    for ns in range(2):
                        nc.tensor.matmul(
                            pv[:, ns * 512:(ns + 1) * 512], eye16_sb[:],
                            v16[:, hn * 1024 + ns * 512: hn * 1024 + (ns + 1) * 512])
                    nc.scalar.copy(v_sb[:, hn * 1024:(hn + 1) * 1024], pv[:])

            # s0 tail: evict second accumulation -> bf16, AllGather, reduce
            s0b = small.tile([16, JD], BF16, tag="s_out", name="s0b")
            nc.scalar.copy(s0b[:], ps0[:])
            ag_s0.append(all_gather(s0b))
            s0 = reduce_partials(ag_s0, "s0")
            squash_broadcast(s0, 1.0 / J)

            # ---- routing iterations ----
            bij = constp.tile([128, G * J], F32)
            a_ps = None

            GC = 4                         # groups per cu chunk
            NCH = G // GC
            CH = GC * JD                   # u elems per chunk per partition

            for it in (1, 2):
                # agreement: tmp = u*v in (d, g, j) order per half (DVE),
                # then sum over d on the tensor engine (64 accumulating
                # identity matmuls into PSUM a[(k,b),(g,j)])
                a_pm = pmain.tile([128, 1024], F32, tag="pmain", name=f"a{it}")
                a_ps = a_pm[:, :G * J]
                for hf in range(NQ):
                    tmp = work.tile([128, GQ * JD], BF16, tag="sc",
                                    name=f"agr{it}_{hf}")
                    u_h = u_sb[:, hf * GQ * JD:(hf + 1) * GQ * JD]
                    nc.vector.tensor_mul(
                        tmp[:].rearrange("p (d g j) -> p g d j", d=D, g=GQ),
                        u_h.rearrange("p (g d j) -> p g d j", g=GQ, d=D),
                        v_sb[:].rearrange("p (d j) -> p d j", d=D)
                            .unsqueeze(1).broadcast_to([128, GQ, D, J]),
                    )
                    for dd in range(D):
                        nc.tensor.matmul(
                            a_ps[:, hf * AQ:(hf + 1) * AQ],
                            eye128_sb[:],
                            tmp[:, dd * AQ:(dd + 1) * AQ],
                            start=(dd == 0), stop=(dd == D - 1),
                            skip_group_check=True,
                        )
                # bij += a ; softmax over j (no max subtraction: logits small)
                if it == 1:
                    nc.vector.tensor_copy(bij[:], a_ps[:])
                else:
                    nc.vector.tensor_add(bij[:], bij[:], a_ps[:])
                eh = small.tile([128, G * J], F32, tag="eh", name="eh")
                nc.scalar.activation(eh[:], bij[:], AF.Exp)
                eh3 = eh[:].rearrange("p (g j) -> p g j", g=G)
                se = small.tile([128, G], F32, tag="se", name="se")
                nc.vector.reduce_sum(se[:], eh3, axis=AX.X)
                re = small.tile([128, G], F32, tag="re", name="re")
                nc.vector.reciprocal(re[:], se[:])
                c_full = small.tile([128, G * J], BF16, tag="c_h",
                                    name="c_full")
                nc.vector.tensor_mul(
                    c_full[:].rearrange("p (g j) -> p g j", g=G), eh3,
                    re[:].unsqueeze(2).broadcast_to([128, G, J]))
                # weighted sum over p: cu = u*c then Delta matmuls -> ps
                ps = pacc.tile([16, JD], F32, tag="pacc", name=f"ps_it{it}")
                for h in range(NCH):
                    u_ch = u_sb[:, h * CH:(h + 1) * CH]
                    u4 = u_ch.rearrange("p (g d j) -> p g d j", g=GC, d=D)
                    cu = work.tile([128, CH], BF16, tag="sc", name=f"cu{it}_{h}")
                    nc.vector.tensor_mul(
                        cu[:].rearrange("p (g d j) -> p g d j", g=GC, d=D),
                        u4,
                        c_full[:, h * GC * J:(h + 1) * GC * J]
                            .rearrange("p (g j) -> p g j", g=GC)
                            .unsqueeze(2).broadcast_to([128, GC, D, J]),
                    )
                    for gg in range(GC):
                        for ns in range(4):
                            nc.tensor.matmul(
                                ps[:, ns * 512:(ns + 1) * 512],
                                delta_sb[:],
                                cu[:, gg * JD + ns * 512: gg * JD + (ns + 1) * 512],
                                start=(h == 0 and gg == 0),
                                stop=(h == NCH - 1 and gg == GC - 1),
                                skip_group_check=True,
                            )
                if it == 1:
                    s1_loc = small.tile([16, JD], BF16, tag="s_out",
                                        name="s1_loc")
                    nc.scalar.copy(s1_loc[:], ps[:])
                    ag_s1 = all_gather(s1_loc)
                    s1 = reduce_partials([ag_s1], "s1")
                    squash_broadcast(s1, 1.0)
                else:
                    s2_sb = small.tile([16, JD], F32, tag="s_loc", name="s2_sb")
                    nc.scalar.copy(s2_sb[:], ps[:])
                    nc.sync.dma_start(out_dram[:], s2_sb[:])

    nc.compile()
    return nc


def pack_inputs(inp, W, b, n_cores: int, n_groups: int):
    """Host-side packing -> per-core in_maps. W columns in (d, j) order."""
    P = inp.shape[1]
    G = n_groups
    ploc = 8 * G
    npair = ploc // 2
    nblk = npair // 2
    assert n_cores * ploc == P

    bf = ml_dtypes.bfloat16
    if b is not None and np.any(b):
        raise NotImplementedError("nonzero bias b is not supported")
    # W[0]: [P, J, E, D] -> [P, E, (D, J)]
    Wt = np.ascontiguousarray(W[0].transpose(0, 2, 3, 1)).reshape(P, E, JD)
    Wp = Wt.reshape(P // 2, 2 * E, JD)
    Wb = Wp.reshape(n_cores, nblk, 2, 2 * E, JD).transpose(0, 1, 3, 2, 4)
    w_dev = np.ascontiguousarray(Wb).reshape(n_cores, nblk, 128, 2 * JD).astype(bf)

    # x: [B, P, E] -> block diag lhsT [c, 128, npair*32]
    inpT = inp.transpose(1, 2, 0)          # [P, E, B]
    arr = inpT.reshape(n_cores, npair, 2, E, B)
    x_dev = np.zeros((n_cores, 2, E, npair, 2, 16), np.float32)
    x_dev[:, 0, :, :, 0, :] = arr[:, :, 0].transpose(0, 2, 1, 3)
    x_dev[:, 1, :, :, 1, :] = arr[:, :, 1].transpose(0, 2, 1, 3)
    x_dev = x_dev.reshape(n_cores, 128, npair * 32).astype(bf)

    in_maps = []
    for c in range(n_cores):
        in_maps.append({"w": w_dev[c], "x": x_dev[c]})
    return in_maps


def squash_np(x):
    s2 = np.sum(x * x, axis=-1, keepdims=True)
    return x * (s2 / (1.0 + s2)) / np.sqrt(s2)


_CACHE = {}


def kernel(inp: np.ndarray, W: np.ndarray, b: np.ndarray) -> np.ndarray:
    n_cores, n_groups = 8, 16
    inp = np.asarray(inp, dtype=np.float32)
    W = np.asarray(W, dtype=np.float32)
    b = np.asarray(b, dtype=np.float32)

    key = (n_cores, n_groups)
    if key not in _CACHE:
        _CACHE[key] = build_program(n_cores, n_groups)
    nc = _CACHE[key]

    in_maps = pack_inputs(inp, W, b, n_cores, n_groups)
    res = run_bass_kernel_spmd(nc, in_maps, core_ids=list(range(n_cores)))
    s2 = np.zeros((16, JD), np.float64)
    for r in res.results:
        s2 += r["out"].astype(np.float64)
    # s layout [16, (d, j)] -> [B, J, D]
    v = squash_np(s2.reshape(B, D, J).transpose(0, 2, 1))
    return v.astype(np.float32)
